# revision 8
# baseline (speedup 1.0000x reference)
"""2-layer LSTM (B=64, T=256, D=1024, H=1536) on 8 TRN2 NeuronCores.

Strategy: 8-way tensor parallelism over the 4*out gate dimension of both
layers (hint's "very large hidden" branch). All weight shards stay resident
in SBUF (~13 MB/core). The input projection x@W_ih0 is fused into the
recurrent matmul groups (x k-tiles accumulate into the same PSUM group as
the h k-tiles, scheduled to run during the AllGather latency window).
Each step does AllGather(h0_shard) and AllGather(h1_shard) across the 8
cores via the collective engine; layer 1 runs one step behind layer 0 so
its matmuls fill the AllGather latency window on the PE.

Per-core layouts (partition dim first):
  - states kept "batch on partitions": gates/c/h tiles are [64, cols]
  - matmul stationary operand is hT / xT k-tiles [128, 64] so the wide
    weight matrices stream through the PE as the moving operand
  - gate column order within a shard is [i | f | o | g] so one sigmoid
    covers [0:3s] and one tanh covers [3s:4s]
  - biases enter via a K=1 matmul with a ones row (start=True of each
    PSUM accumulation group), so ACT reads gate preactivations straight
    from PSUM.
"""

import numpy as np

import concourse.bass as bass
import concourse.bacc as bacc
import concourse.mybir as mybir
from concourse import masks
from concourse import bass_utils
from concourse.tile import TileContext

F32 = mybir.dt.float32
AF = mybir.ActivationFunctionType

B = 64
T = 256
D = 1024  # input size / layer-1 output size
H = 1536  # layer-0 hidden size
HSH = H // 8   # 192 layer-0 hidden units per core
DSH = D // 8   # 128 layer-1 hidden units per core
NK_X = D // 128    # 8 k-tiles for x -> layer0
NK_H0 = H // 128   # 12 k-tiles for h0
NK_H1 = D // 128   # 8 k-tiles for h1
G0 = 4 * HSH   # 768 gate columns per core, layer 0
G1 = 4 * DSH   # 512 gate columns per core, layer 1


def build_program(steps: int = T):
    nc = bacc.Bacc(
        "TRN2", target_bir_lowering=False, debug=False, num_devices=8
    )

    # ---- external I/O (per-core contents supplied via in_maps) ----
    w0_d = nc.dram_tensor("w0", [NK_X, 128, G0], F32, kind="ExternalInput")
    u0_d = nc.dram_tensor("u0", [NK_H0, 128, G0], F32, kind="ExternalInput")
    w1_d = nc.dram_tensor("w1", [NK_H0, 128, G1], F32, kind="ExternalInput")
    u1_d = nc.dram_tensor("u1", [NK_H1, 128, G1], F32, kind="ExternalInput")
    b0_d = nc.dram_tensor("b0r", [1, G0], F32, kind="ExternalInput")
    b1_d = nc.dram_tensor("b1r", [1, G1], F32, kind="ExternalInput")
    xs_d = nc.dram_tensor("xs", [steps, 128, NK_X * 64], F32, kind="ExternalInput")
    h0i_d = nc.dram_tensor("h0i", [128, NK_H0 * 64], F32, kind="ExternalInput")
    h1i_d = nc.dram_tensor("h1i", [128, NK_H1 * 64], F32, kind="ExternalInput")
    c0i_d = nc.dram_tensor("c0i", [64, HSH], F32, kind="ExternalInput")
    c1i_d = nc.dram_tensor("c1i", [64, DSH], F32, kind="ExternalInput")

    y_d = nc.dram_tensor("y", [steps, 64, DSH], F32, kind="ExternalOutput")
    hn0_d = nc.dram_tensor("hn0", [64, HSH], F32, kind="ExternalOutput")
    cn0_d = nc.dram_tensor("cn0", [64, HSH], F32, kind="ExternalOutput")
    hn1_d = nc.dram_tensor("hn1", [64, DSH], F32, kind="ExternalOutput")
    cn1_d = nc.dram_tensor("cn1", [64, DSH], F32, kind="ExternalOutput")

    rg = [list(range(8))]

    with TileContext(nc) as tc:
        with (
            tc.tile_pool(name="const", bufs=1) as const,
            tc.tile_pool(name="state", bufs=1) as state,
            tc.tile_pool(name="hg", bufs=2) as hgp,         # gathered hT tiles
            tc.tile_pool(name="xt", bufs=2) as xtp,
            tc.tile_pool(name="act", bufs=2) as actp,
            tc.tile_pool(name="cell", bufs=2) as cellp,
            tc.tile_pool(name="ps0", bufs=2, space="PSUM") as ps0p,
            tc.tile_pool(name="ps1", bufs=1, space="PSUM") as ps1p,
            tc.tile_pool(name="pstr", bufs=2, space="PSUM") as pstrp,
            tc.tile_pool(name="dram", bufs=2, space="DRAM") as dramp,
        ):
            # ---- resident weights ----
            w0 = const.tile([128, NK_X, G0], F32)
            u0 = const.tile([128, NK_H0, G0], F32)
            w1 = const.tile([128, NK_H0, G1], F32)
            u1 = const.tile([128, NK_H1, G1], F32)
            nc.sync.dma_start(w0[:], w0_d.ap().rearrange("k p g -> p k g"))
            nc.sync.dma_start(u0[:], u0_d.ap().rearrange("k p g -> p k g"))
            nc.sync.dma_start(w1[:], w1_d.ap().rearrange("k p g -> p k g"))
            nc.sync.dma_start(u1[:], u1_d.ap().rearrange("k p g -> p k g"))
            b0r = const.tile([1, G0], F32)
            b1r = const.tile([1, G1], F32)
            nc.sync.dma_start(b0r[:], b0_d.ap())
            nc.sync.dma_start(b1r[:], b1_d.ap())
            ones = const.tile([1, 64], F32)
            nc.gpsimd.memset(ones[:], 1.0)
            ident = const.tile([64, 64], F32)
            masks.make_identity(nc, ident[:])

            # ---- state ----
            c0 = state.tile([64, HSH], F32)
            c1 = state.tile([64, DSH], F32)
            nc.sync.dma_start(c0[:], c0i_d.ap())
            nc.sync.dma_start(c1[:], c1i_d.ap())

            h0g = hgp.tile([128, NK_H0, 64], F32, name="h0g")
            nc.sync.dma_start(h0g[:], h0i_d.ap().rearrange("p (k b) -> p k b", k=NK_H0))
            h1g = hgp.tile([128, NK_H1, 64], F32, name="h1g")
            nc.sync.dma_start(h1g[:], h1i_d.ap().rearrange("p (k b) -> p k b", k=NK_H1))

            def l0_xpart(t):
                """bias + x@W_ih0 for step t: opens the psum group for step t."""
                ps0 = ps0p.tile([64, G0], F32, name="ps0")
                xt = xtp.tile([128, NK_X, 64], F32, name="xt")
                nc.sync.dma_start(
                    xt[:], xs_d.ap()[t].rearrange("p (k b) -> p k b", k=NK_X)
                )
                nc.tensor.matmul(ps0[:, 0:512], ones[:], b0r[:, 0:512],
                                 start=True, stop=False)
                nc.tensor.matmul(ps0[:, 512:G0], ones[:], b0r[:, 512:G0],
                                 start=True, stop=False)
                for k in range(NK_X):
                    nc.tensor.matmul(ps0[:, 0:512], xt[:, k, :], w0[:, k, 0:512],
                                     start=False, stop=False)
                    nc.tensor.matmul(ps0[:, 512:G0], xt[:, k, :], w0[:, k, 512:G0],
                                     start=False, stop=False)
                return ps0

            ps0_cur = l0_xpart(0)
            h0new = None
            h1new = None

            for t in range(steps):
                # -- layer-0 recurrent matmuls for step t (wait on h0g) --
                for k in range(NK_H0):
                    last = k == NK_H0 - 1
                    nc.tensor.matmul(ps0_cur[:, 0:512], h0g[:, k, :],
                                     u0[:, k, 0:512], start=False, stop=last)
                    nc.tensor.matmul(ps0_cur[:, 512:G0], h0g[:, k, :],
                                     u0[:, k, 512:G0], start=False, stop=last)

                # -- open next step's group early: PE filler work --
                ps0_nxt = l0_xpart(t + 1) if t + 1 < steps else None

                # -- layer-0 cell (gate order i|f|o|g; s = HSH) --
                act0 = actp.tile([64, G0], F32, name="act0")
                nc.scalar.activation(act0[:, 0:3 * HSH], ps0_cur[:, 0:3 * HSH],
                                     AF.Sigmoid)
                nc.scalar.activation(act0[:, 3 * HSH:G0], ps0_cur[:, 3 * HSH:G0],
                                     AF.Tanh)
                m1 = cellp.tile([64, HSH], F32, name="m1")
                m2 = cellp.tile([64, HSH], F32, name="m2")
                nc.vector.tensor_mul(m1[:], act0[:, HSH:2 * HSH], c0[:])
                nc.vector.tensor_mul(m2[:], act0[:, 0:HSH], act0[:, 3 * HSH:G0])
                nc.vector.tensor_add(c0[:], m1[:], m2[:])
                tc0 = cellp.tile([64, HSH], F32, name="tc0")
                nc.scalar.activation(tc0[:], c0[:], AF.Tanh)
                h0new = cellp.tile([64, HSH], F32, name="h0new")
                nc.vector.tensor_mul(h0new[:], act0[:, 2 * HSH:3 * HSH], tc0[:])

                # -- transpose h0new -> [HSH, 64] and AllGather --
                tr0a = pstrp.tile([128, 64], F32, name="tr0a", tag="tr")
                tr0b = pstrp.tile([64, 64], F32, name="tr0b", tag="tr")
                nc.tensor.transpose(tr0a[:], h0new[:, 0:128], ident[:])
                nc.tensor.transpose(tr0b[:], h0new[:, 128:HSH], ident[:])
                str0 = actp.tile([128, 128], F32, name="str0")
                nc.vector.tensor_copy(str0[:, 0:64], tr0a[:])
                nc.vector.tensor_copy(str0[0:64, 64:128], tr0b[:])
                bounce0 = dramp.tile([HSH, 64], F32, name="bounce0")
                nc.sync.dma_start(bounce0[0:128], str0[:, 0:64])
                nc.sync.dma_start(bounce0[128:HSH], str0[0:64, 64:128])
                shared0 = dramp.tile([H, 64], F32, name="shared0",
                                     addr_space="Shared")
                nc.gpsimd.collective_compute(
                    "AllGather", mybir.AluOpType.bypass, replica_groups=rg,
                    ins=[bounce0.opt()], outs=[shared0.opt()],
                )
                h0g_nxt = hgp.tile([128, NK_H0, 64], F32, name="h0g")
                nc.sync.dma_start(
                    h0g_nxt[:], shared0.rearrange("(k p) b -> p k b", p=128)
                )

                # -- layer 1 for step t-1 (h0g holds gathered h0_{t-1}) --
                # runs on PE inside the AllGather(h0_t) latency window
                if t >= 1:
                    ps1 = ps1p.tile([64, G1], F32, name="ps1")
                    nc.tensor.matmul(ps1[:], ones[:], b1r[:], start=True, stop=False)
                    for k in range(NK_H0):
                        nc.tensor.matmul(ps1[:], h0g[:, k, :], w1[:, k, :],
                                         start=False, stop=False)
                    for k in range(NK_H1):
                        nc.tensor.matmul(ps1[:], h1g[:, k, :], u1[:, k, :],
                                         start=False, stop=k == NK_H1 - 1)

                    act1 = actp.tile([64, G1], F32, name="act1")
                    nc.scalar.activation(act1[:, 0:3 * DSH], ps1[:, 0:3 * DSH],
                                         AF.Sigmoid)
                    nc.scalar.activation(act1[:, 3 * DSH:G1], ps1[:, 3 * DSH:G1],
                                         AF.Tanh)
                    n1 = cellp.tile([64, DSH], F32, name="n1")
                    n2 = cellp.tile([64, DSH], F32, name="n2")
                    nc.vector.tensor_mul(n1[:], act1[:, DSH:2 * DSH], c1[:])
                    nc.vector.tensor_mul(n2[:], act1[:, 0:DSH], act1[:, 3 * DSH:G1])
                    nc.vector.tensor_add(c1[:], n1[:], n2[:])
                    tc1 = cellp.tile([64, DSH], F32, name="tc1")
                    nc.scalar.activation(tc1[:], c1[:], AF.Tanh)
                    h1new = cellp.tile([64, DSH], F32, name="h1new")
                    nc.vector.tensor_mul(h1new[:], act1[:, 2 * DSH:3 * DSH], tc1[:])
                    nc.sync.dma_start(y_d.ap()[t - 1], h1new[:])

                    if t < steps:  # gather h1_{t-1} for step t's layer 1
                        tr1 = pstrp.tile([128, 64], F32, name="tr1", tag="tr")
                        nc.tensor.transpose(tr1[:], h1new[:, 0:128], ident[:])
                        str1 = actp.tile([128, 64], F32, name="str1")
                        nc.vector.tensor_copy(str1[:], tr1[:])
                        bounce1 = dramp.tile([DSH, 64], F32, name="bounce1")
                        nc.sync.dma_start(bounce1[:], str1[:])
                        shared1 = dramp.tile([D, 64], F32, name="shared1",
                                             addr_space="Shared")
                        nc.gpsimd.collective_compute(
                            "AllGather", mybir.AluOpType.bypass, replica_groups=rg,
                            ins=[bounce1.opt()], outs=[shared1.opt()],
                        )
                        h1g = hgp.tile([128, NK_H1, 64], F32, name="h1g")
                        nc.sync.dma_start(
                            h1g[:], shared1.rearrange("(k p) b -> p k b", p=128)
                        )

                h0g = h0g_nxt
                ps0_cur = ps0_nxt

            # ---- drain: layer 1 for the final step ----
            t = steps
            ps1 = ps1p.tile([64, G1], F32, name="ps1")
            nc.tensor.matmul(ps1[:], ones[:], b1r[:], start=True, stop=False)
            for k in range(NK_H0):
                nc.tensor.matmul(ps1[:], h0g[:, k, :], w1[:, k, :],
                                 start=False, stop=False)
            for k in range(NK_H1):
                nc.tensor.matmul(ps1[:], h1g[:, k, :], u1[:, k, :],
                                 start=False, stop=k == NK_H1 - 1)
            act1 = actp.tile([64, G1], F32, name="act1")
            nc.scalar.activation(act1[:, 0:3 * DSH], ps1[:, 0:3 * DSH], AF.Sigmoid)
            nc.scalar.activation(act1[:, 3 * DSH:G1], ps1[:, 3 * DSH:G1], AF.Tanh)
            n1 = cellp.tile([64, DSH], F32, name="n1")
            n2 = cellp.tile([64, DSH], F32, name="n2")
            nc.vector.tensor_mul(n1[:], act1[:, DSH:2 * DSH], c1[:])
            nc.vector.tensor_mul(n2[:], act1[:, 0:DSH], act1[:, 3 * DSH:G1])
            nc.vector.tensor_add(c1[:], n1[:], n2[:])
            tc1 = cellp.tile([64, DSH], F32, name="tc1")
            nc.scalar.activation(tc1[:], c1[:], AF.Tanh)
            h1new = cellp.tile([64, DSH], F32, name="h1new")
            nc.vector.tensor_mul(h1new[:], act1[:, 2 * DSH:3 * DSH], tc1[:])
            nc.sync.dma_start(y_d.ap()[t - 1], h1new[:])

            # ---- final states ----
            nc.sync.dma_start(hn0_d.ap(), h0new[:])
            nc.sync.dma_start(cn0_d.ap(), c0[:])
            nc.sync.dma_start(hn1_d.ap(), h1new[:])
            nc.sync.dma_start(cn1_d.ap(), c1[:])

    nc.compile()
    return nc


def shard_cols(w, out_dim, c, sh):
    """Slice gate columns for core c, reordering gates i,f,g,o -> i,f,o,g."""
    s, e = c * sh, (c + 1) * sh
    return np.concatenate(
        [w[..., 0 * out_dim + s:0 * out_dim + e],
         w[..., 1 * out_dim + s:1 * out_dim + e],
         w[..., 3 * out_dim + s:3 * out_dim + e],
         w[..., 2 * out_dim + s:2 * out_dim + e]], axis=-1,
    )


def make_in_maps(x, h0_0, c0_0, h0_1, c0_1, W_ih0, W_hh0, b0, W_ih1, W_hh1, b1,
                 steps=T):
    xs = np.ascontiguousarray(
        x[:, :steps, :].transpose(1, 2, 0)                 # [t, d, b]
        .reshape(steps, NK_X, 128, B).transpose(0, 2, 1, 3) # [t, p, k, b]
        .reshape(steps, 128, NK_X * B)
    ).astype(np.float32)
    h0i = np.ascontiguousarray(
        h0_0.T.reshape(NK_H0, 128, B).transpose(1, 0, 2).reshape(128, NK_H0 * B)
    ).astype(np.float32)
    h1i = np.ascontiguousarray(
        h0_1.T.reshape(NK_H1, 128, B).transpose(1, 0, 2).reshape(128, NK_H1 * B)
    ).astype(np.float32)

    in_maps = []
    for c in range(8):
        in_maps.append({
            "w0": np.ascontiguousarray(
                shard_cols(W_ih0, H, c, HSH).reshape(NK_X, 128, G0)),
            "u0": np.ascontiguousarray(
                shard_cols(W_hh0, H, c, HSH).reshape(NK_H0, 128, G0)),
            "w1": np.ascontiguousarray(
                shard_cols(W_ih1, D, c, DSH).reshape(NK_H0, 128, G1)),
            "u1": np.ascontiguousarray(
                shard_cols(W_hh1, D, c, DSH).reshape(NK_H1, 128, G1)),
            "b0r": np.ascontiguousarray(shard_cols(b0[None, :], H, c, HSH)),
            "b1r": np.ascontiguousarray(shard_cols(b1[None, :], D, c, DSH)),
            "xs": xs,
            "h0i": h0i,
            "h1i": h1i,
            "c0i": np.ascontiguousarray(c0_0[:, c * HSH:(c + 1) * HSH]),
            "c1i": np.ascontiguousarray(c0_1[:, c * DSH:(c + 1) * DSH]),
        })
    return in_maps


def assemble_outputs(results, steps=T):
    """results: list of 8 per-core dicts -> reference-shaped outputs."""
    y = np.concatenate([r["y"] for r in results], axis=2)   # [t, 64, 1024]
    outputs = np.ascontiguousarray(y.transpose(1, 0, 2))    # [64, t, 1024]
    hn0 = np.concatenate([r["hn0"] for r in results], axis=1)
    cn0 = np.concatenate([r["cn0"] for r in results], axis=1)
    hn1 = np.concatenate([r["hn1"] for r in results], axis=1)
    cn1 = np.concatenate([r["cn1"] for r in results], axis=1)
    return outputs, hn0, cn0, hn1, cn1


def kernel(x, h0_0, c0_0, h0_1, c0_1, W_ih0, W_hh0, b0, W_ih1, W_hh1, b1,
           steps=T, **run_kwargs):
    args = [np.asarray(a, dtype=np.float32)
            for a in (x, h0_0, c0_0, h0_1, c0_1,
                      W_ih0, W_hh0, b0, W_ih1, W_hh1, b1)]
    nc = build_program(steps)
    in_maps = make_in_maps(*args, steps=steps)
    res = bass_utils.run_bass_kernel_spmd(
        nc, in_maps, core_ids=list(range(8)), **run_kwargs
    )
    out = assemble_outputs(res.results, steps=steps)
    kernel.last_results = res
    return out
